# revision 23
# baseline (speedup 1.0000x reference)
"""Multi-head attention (RoPE + mask + softmax) Trainium2 Bass kernel.

Sharding: 8 cores = 2 batches x 4 head-groups. Core c handles batch c//4,
local heads 4*(c%4) .. +4 (tensor-parallel on heads; Wq/Wk/Wv column-sharded,
Wo row-sharded; per-core partial outputs summed on host).

v3: all-bf16 matmul operands, f32 PSUM. Paired K=64 score matmuls (rows
0-63 / 64-127 concurrently). exp on ACT; mask-mul split DVE/gpsimd;
denominator: ones-column row of AV psum -> DVE reciprocal -> gpsimd
partition_broadcast -> fused DVE normalize. PSUM: 2x [128,1024] "work"
slots (scores/proj/outproj) + 2x [65,1024] "avp". Projections emit both
head-pair halves into one [128,1024] psum so RoPE runs 1024-wide.
v/q/k-proj + outproj interleaved into attention m-loops as PE filler;
mask chunks DMA'd just-in-time; output written straight from PSUM.
"""
import sys
sys.path.insert(0, '/opt/trn_rl_repo')
import math
import numpy as np
import ml_dtypes

import concourse.bass as bass
import concourse.mybir as mybir
import concourse.tile as tile
from concourse import bacc
from concourse.bass_utils import run_bass_kernel_spmd

F32 = mybir.dt.float32
BF16 = mybir.dt.bfloat16

S = 2048
DIM = 1024
HEAD_DIM = 64
N_CORES = 8
KC = DIM // 128          # 8 contraction chunks for projections
MT = S // 128            # 16 k-chunks in attention
QB = 1024                # q-block width
ROPE_THETA = 10000.0

_BUILT = None


def build_bass(dbg=False):
    nc = bacc.Bacc("TRN2", target_bir_lowering=False, debug=False)

    qT = nc.dram_tensor("qT", [4, 128, KC, 512], BF16, kind="ExternalInput").ap()
    kT = nc.dram_tensor("kT", [4, 128, KC, 512], BF16, kind="ExternalInput").ap()
    vT = nc.dram_tensor("vT", [MT, 128, KC, 128], BF16, kind="ExternalInput").ap()
    wq = nc.dram_tensor("wq", [128, KC, 256], BF16, kind="ExternalInput").ap()
    wk = nc.dram_tensor("wk", [128, KC, 256], BF16, kind="ExternalInput").ap()
    wv = nc.dram_tensor("wv", [128, KC, 256], BF16, kind="ExternalInput").ap()
    wo = nc.dram_tensor("wo", [64, 4, DIM], BF16, kind="ExternalInput").ap()
    cosT = nc.dram_tensor("cosT", [128, S], BF16, kind="ExternalInput").ap()
    sinT = nc.dram_tensor("sinT", [128, S], BF16, kind="ExternalInput").ap()
    maskT = nc.dram_tensor("maskT", [128, MT, S], BF16, kind="ExternalInput").ap()
    out_part = nc.dram_tensor("out_part", [S, DIM], BF16, kind="ExternalOutput").ap()
    if dbg:
        qhT_d = nc.dram_tensor("qhT_d", [128, 2, S], BF16,
                               kind="ExternalOutput").ap()
        khT_d = nc.dram_tensor("khT_d", [128, 2, S], BF16,
                               kind="ExternalOutput").ap()
        vh_d = nc.dram_tensor("vh_d", [128, MT, 4 * 65], BF16,
                              kind="ExternalOutput").ap()
        at_d = nc.dram_tensor("at_d", [128, 8, QB], BF16,
                              kind="ExternalOutput").ap()
        outT_d = nc.dram_tensor("outT_d", [64, 4, S], BF16,
                                kind="ExternalOutput").ap()

    with tile.TileContext(nc) as tc:
        with tc.tile_pool(name="persist", bufs=1) as persist, \
             tc.tile_pool(name="ps", bufs=2, space="PSUM") as ps, \
             tc.tile_pool(name="xts", bufs=4) as xts, \
             tc.tile_pool(name="vx", bufs=4) as vxp, \
             tc.tile_pool(name="rope", bufs=2) as rope, \
             tc.tile_pool(name="attn", bufs=3) as attnp, \
             tc.tile_pool(name="atmp", bufs=8) as atmp, \
             tc.tile_pool(name="dn", bufs=2) as dnp, \
             tc.tile_pool(name="outp", bufs=2) as outp:

            qhT = persist.tile([128, 2, S], BF16)     # [chunk-part, hp, s]
            khT = persist.tile([128, 2, S], BF16)
            vh = persist.tile([128, MT, 4 * 65], BF16)
            outT = persist.tile([64, 4, S], BF16)
            wo_sb = persist.tile([64, 4, DIM], BF16)
            wq_sb = persist.tile([128, KC, 256], BF16)
            wk_sb = persist.tile([128, KC, 256], BF16)
            wv_sb = persist.tile([128, KC, 256], BF16)
            cos_sb = persist.tile([128, 2, S], BF16)  # duplicated per head-pair
            sin_sb = persist.tile([128, 2, S], BF16)
            mk = persist.tile([128, MT, QB], BF16)

            # ---- weight DMAs first (sync), tables + early mask (scalar) ----
            nc.sync.dma_start(out=wk_sb, in_=wk)
            nc.sync.dma_start(out=wq_sb, in_=wq)
            nc.sync.dma_start(out=wv_sb, in_=wv)
            for hp in range(2):
                nc.scalar.dma_start(out=cos_sb[:, hp, :], in_=cosT)
                nc.scalar.dma_start(out=sin_sb[:, hp, :], in_=sinT)
            for m in range(4):
                nc.scalar.dma_start(out=mk[:, m, :], in_=maskT[:, m, 0:QB])
            # ones column for the denominator rows of vh
            nc.vector.memset(
                vh.rearrange("p m (h x) -> p m h x", x=65)[:, :, :, 64:65], 1.0)

            _xcache = {}

            def proj_rope(xdram, w_sb, dstT, sblk):
                """One sblk projection (both head-pairs) with fused RoPE."""
                key = (id(xdram), sblk)
                if key not in _xcache:
                    halves = []
                    for kh2 in range(2):
                        xh = xts.tile([128, 4, 512], BF16, tag="xts",
                                      name=f"x_{id(xdram) & 0xffff}_{sblk}_{kh2}")
                        nc.sync.dma_start(out=xh, in_=xdram[sblk, :, kh2 * 4:(kh2 + 1) * 4, :])
                        halves.append(xh)
                    _xcache[key] = halves
                halves = _xcache[key]
                ss = slice(sblk * 512, (sblk + 1) * 512)
                psum = ps.tile([128, 2, 512], F32, tag="work",
                               name=f"pj_{id(xdram) & 0xffff}_{sblk}")
                for hp in range(2):
                    for kc in range(KC):
                        nc.tensor.matmul(
                            psum[:, hp, :],
                            lhsT=w_sb[:, kc, hp * 128:(hp + 1) * 128],
                            rhs=halves[kc // 4][:, kc % 4, :],
                            start=(kc == 0), stop=(kc == KC - 1))
                t = rope.tile([128, 2, 512], BF16, tag="t")
                u = rope.tile([128, 2, 512], BF16, tag="u")
                nc.vector.tensor_mul(t, psum, cos_sb[:, :, ss])
                nc.vector.tensor_mul(u, psum, sin_sb[:, :, ss])
                us = rope.tile([128, 2, 512], BF16, tag="us")
                for blk in range(4):
                    a, b2 = blk * 32, (blk ^ 1) * 32
                    nc.sync.dma_start(out=us[a:a + 32, :, :], in_=u[b2:b2 + 32, :, :])
                nc.vector.tensor_add(dstT[:, :, ss], t, us)

            def proj_rope_hp(xdram, w_sb, dstT, sblk, hp):
                """Half projection unit (one head-pair) for finer fillers."""
                key = (id(xdram), sblk)
                if key not in _xcache:
                    halves = []
                    for kh2 in range(2):
                        xh = xts.tile([128, 4, 512], BF16, tag="xts",
                                      name=f"xh_{id(xdram) & 0xffff}_{sblk}_{kh2}")
                        nc.sync.dma_start(out=xh,
                                          in_=xdram[sblk, :, kh2 * 4:(kh2 + 1) * 4, :])
                        halves.append(xh)
                    _xcache[key] = halves
                halves = _xcache[key]
                ss = slice(sblk * 512, (sblk + 1) * 512)
                psum = ps.tile([128, 512], F32, tag="work",
                               name=f"pjh_{id(xdram) & 0xffff}_{sblk}_{hp}")
                for kc in range(KC):
                    nc.tensor.matmul(
                        psum,
                        lhsT=w_sb[:, kc, hp * 128:(hp + 1) * 128],
                        rhs=halves[kc // 4][:, kc % 4, :],
                        start=(kc == 0), stop=(kc == KC - 1))
                t = rope.tile([128, 512], BF16, tag="t",
                              name=f"th_{id(xdram) & 0xffff}_{sblk}_{hp}")
                u = rope.tile([128, 512], BF16, tag="u",
                              name=f"uh_{id(xdram) & 0xffff}_{sblk}_{hp}")
                nc.vector.tensor_mul(t, psum, cos_sb[:, hp, ss])
                nc.vector.tensor_mul(u, psum, sin_sb[:, hp, ss])
                us = rope.tile([128, 512], BF16, tag="us",
                               name=f"ush_{id(xdram) & 0xffff}_{sblk}_{hp}")
                for blk in range(4):
                    a, b2 = blk * 32, (blk ^ 1) * 32
                    nc.sync.dma_start(out=us[a:a + 32, :], in_=u[b2:b2 + 32, :])
                nc.vector.tensor_add(dstT[:, hp, ss], t, us)

            def vproj(sc):
                v_sb = vxp.tile([128, KC, 128], BF16, tag="vts", name=f"v_{sc}")
                nc.sync.dma_start(out=v_sb, in_=vT[sc])
                psum = ps.tile([128, 256], F32, tag="work", name=f"vp_{sc}")
                for kc in range(KC):
                    nc.tensor.matmul(
                        psum, lhsT=v_sb[:, kc, :], rhs=wv_sb[:, kc, :],
                        start=(kc == 0), stop=(kc == KC - 1))
                nc.vector.tensor_copy(
                    vh[:, sc, :].rearrange("p (h x) -> p h x", x=65)[:, :, 0:64],
                    psum.rearrange("p (h x) -> p h x", x=64))

            def outproj(sc):
                """Output projection for one 128-row s-chunk, PSUM -> DRAM."""
                wps = ps.tile([128, 2, 512], F32, tag="work", name=f"op_{sc}")
                for nb in range(2):
                    for h in range(4):
                        nc.tensor.matmul(
                            wps[:, nb, :],
                            lhsT=outT[0:64, h, sc * 128:(sc + 1) * 128],
                            rhs=wo_sb[0:64, h, nb * 512:(nb + 1) * 512],
                            start=(h == 0), stop=(h == 3))
                oc = outp.tile([128, DIM], BF16, tag="oc", name=f"oc_{sc}")
                nc.scalar.copy(oc, wps.rearrange("p a b -> p (a b)"))
                nc.sync.dma_start(
                    out=out_part[sc * 128:(sc + 1) * 128, :], in_=oc)

            def attention_unit(qb, hp, fillers, dump=None):
                """Attention for (q-block, head-pair): 16 m-chunks + normalize.

                fillers: dict m -> list of callables emitted at the top of
                that m-iteration (PE filler work + JIT DMAs).
                """
                qs = slice(qb * QB, (qb + 1) * QB)
                avp = [ps.tile([65, QB], F32, tag="avp", name=f"avp{qb}_{hp}_{i}")
                       for i in range(2)]
                atm_q = []

                def av_pair(mm):
                    atm2 = atm_q.pop(0)
                    for h2 in range(2):
                        h = 2 * hp + h2
                        for q2 in range(2):
                            nc.tensor.matmul(
                                avp[h2][:, q2 * 512:(q2 + 1) * 512],
                                lhsT=vh[:, mm, h * 65:(h + 1) * 65],
                                rhs=atm2[h2][:, q2 * 512:(q2 + 1) * 512],
                                start=(mm == 0), stop=(mm == MT - 1))

                for m in range(MT):
                    for f in fillers.get(m, ()):
                        if getattr(f, "is_dma", False):
                            f()
                    sps_t = [None, None]
                    for h2 in range(2):
                        sps_t[h2] = ps.tile([128, QB], F32, tag="work",
                                            name=f"sps_{qb}_{hp}_{m}_{h2}")
                    for q2 in range(2):
                        for h2 in range(2):
                            hb = slice(h2 * 64, (h2 + 1) * 64)
                            nc.tensor.matmul(
                                sps_t[h2][:, q2 * 512:(q2 + 1) * 512],
                                lhsT=khT[hb, hp, m * 128:(m + 1) * 128],
                                rhs=qhT[hb, hp, qs][:, q2 * 512:(q2 + 1) * 512],
                                start=True, stop=True)
                    atm2 = [None, None]
                    for h2 in range(2):
                        at = attnp.tile([128, QB], BF16, tag="at")
                        nc.scalar.activation(
                            at, sps_t[h2], mybir.ActivationFunctionType.Exp,
                            scale=1.0 / math.sqrt(HEAD_DIM))
                        atm2[h2] = atmp.tile([128, QB], BF16, tag="atm",
                                              name=f"atm_{qb}_{hp}_{m}_{h2}")
                        eng = nc.gpsimd if (2 * m + h2) % 4 == 3 else nc.vector
                        eng.tensor_mul(atm2[h2], at, mk[:, m, :])
                        if dump is not None and m < 4:
                            nc.sync.dma_start(out=dump[:, 2 * m + h2, :],
                                              in_=atm2[h2])
                    atm_q.append(atm2)
                    for f in fillers.get(m, ()):
                        if not getattr(f, "is_dma", False):
                            f()
                    if m >= 3:
                        av_pair(m - 3)
                av_pair(MT - 3)
                av_pair(MT - 2)
                av_pair(MT - 1)
                # normalize: recip of denominator row, broadcast, fused evict
                rdnb = [None, None]
                for h2 in range(2):
                    row = dnp.tile([1, QB], F32, tag="row",
                                   name=f"row{qb}_{hp}_{h2}")
                    nc.vector.tensor_copy(row, avp[h2][64:65, :])
                    rf = dnp.tile([1, QB], F32, tag="rf",
                                  name=f"rf{qb}_{hp}_{h2}")
                    nc.vector.reciprocal_approx_fast(rf, row)
                    rdnb[h2] = dnp.tile([64, QB], F32, tag="rdnb",
                                        name=f"rdnb{qb}_{hp}_{h2}")
                    nc.gpsimd.partition_broadcast(rdnb[h2], rf)
                for h2 in range(2):
                    h = 2 * hp + h2
                    nc.vector.tensor_mul(outT[0:64, h, qs], avp[h2][0:64, :],
                                         rdnb[h2])

            # ---------------- emission schedule ----------------
            for sblk in (0, 1):
                proj_rope(kT, wk_sb, khT, sblk)
            for sblk in (0, 1):
                proj_rope(qT, wq_sb, qhT, sblk)

            # qb0-hp0: fill with k sblk2/3, all v, JIT mask DMAs
            f0 = {m: [] for m in range(MT)}
            f0[0].append(lambda: vproj(0))
            f0[0].append(lambda: vproj(1))
            f0[1].append(lambda: proj_rope(kT, wk_sb, khT, 2))
            f0[3].append(lambda: proj_rope(kT, wk_sb, khT, 3))
            for m in range(MT):
                if m + 2 < MT:
                    f0[m].append(lambda sc=m + 2: vproj(sc))
                if m + 4 < MT:
                    _d = lambda mm=m + 4: nc.scalar.dma_start(
                        out=mk[:, mm, :], in_=maskT[:, mm, 0:QB])
                    _d.is_dma = True
                    f0[m].append(_d)
            attention_unit(0, 0, f0, dump=at_d if dbg else None)

            # qb0-hp1: fill with q sblk2/3, wo DMA
            f1 = {m: [] for m in range(MT)}
            _dwo = lambda: nc.scalar.dma_start(out=wo_sb, in_=wo)
            _dwo.is_dma = True
            f1[0].append(_dwo)
            f1[2].append(lambda: proj_rope_hp(qT, wq_sb, qhT, 2, 0))
            f1[5].append(lambda: proj_rope_hp(qT, wq_sb, qhT, 2, 1))
            f1[8].append(lambda: proj_rope_hp(qT, wq_sb, qhT, 3, 0))
            f1[11].append(lambda: proj_rope_hp(qT, wq_sb, qhT, 3, 1))
            for i, m in enumerate((12, 13, 14, 15)):
                _d1 = lambda mm=i: nc.scalar.dma_start(
                    out=mk[:, mm, :], in_=maskT[:, mm, QB:S])
                _d1.is_dma = True
                f1[m].append(_d1)
            attention_unit(0, 1, f1)

            # qb1: fill with outproj of qb0 rows
            f2 = {m: [] for m in range(MT)}
            for i, m in enumerate((1, 4, 8, 12)):
                f2[m].append(lambda sc=i: outproj(sc))
            for m in range(MT):
                if m + 4 < MT:
                    _d2 = lambda mm=m + 4: nc.scalar.dma_start(
                        out=mk[:, mm, :], in_=maskT[:, mm, QB:S])
                    _d2.is_dma = True
                    f2[m].append(_d2)
            attention_unit(1, 0, f2)
            f3 = {m: [] for m in range(MT)}
            for i, m in enumerate((1, 4, 8, 12)):
                f3[m].append(lambda sc=4 + i: outproj(sc))
            attention_unit(1, 1, f3)
            for sc in range(8, MT):
                outproj(sc)

            if dbg:
                nc.sync.dma_start(out=qhT_d, in_=qhT)
                nc.sync.dma_start(out=khT_d, in_=khT)
                nc.sync.dma_start(out=vh_d, in_=vh)
                nc.sync.dma_start(out=outT_d, in_=outT)

    nc.compile()
    return nc


def _rope_perm_cols():
    """Column permutation of the 256-wide W slice for one core's 4 heads.

    Chunk c (0,1) holds local heads 2c, 2c+1 as rows
    [hA_even(32) | hA_odd(32) | hB_even(32) | hB_odd(32)].
    """
    cols = []
    for c in range(2):
        for j2 in range(2):          # which head within the chunk
            head = 2 * c + j2
            for blk in range(2):     # 0: even dims, 1: odd dims
                for i in range(32):
                    cols.append(head * 64 + 2 * i + blk)
    return np.array(cols)


def _cos_sin_tables():
    inv_freq = 1.0 / (ROPE_THETA ** (np.arange(0, HEAD_DIM, 2, dtype=np.float64)
                                     / HEAD_DIM))          # [32]
    ang = np.arange(S, dtype=np.float64)[None, :] * inv_freq[:, None]  # [32, S]
    cos32 = np.cos(ang).astype(np.float32)
    sin32 = np.sin(ang).astype(np.float32)
    cosT = np.tile(cos32, (4, 1))                           # [128, S]
    # sign: +sin at even-dim rows (blocks 0, 2), -sin at odd-dim rows (1, 3)
    sinT = np.concatenate([sin32, -sin32, sin32, -sin32], axis=0)
    return np.ascontiguousarray(cosT), np.ascontiguousarray(sinT)


def _tile_xT(xT):
    # [1024, 2048] -> [4 sblk, 128 part, 8 kc, 512]
    return np.ascontiguousarray(
        xT.reshape(KC, 128, 4, 512).transpose(2, 1, 0, 3))


def _tile_vT(vT):
    # [1024, 2048] -> [16 sc, 128 part, 8 kc, 128]
    return np.ascontiguousarray(
        vT.reshape(KC, 128, MT, 128).transpose(2, 1, 0, 3))


def _tile_w(w):
    # [1024, 256] -> [128, 8, 256]
    return np.ascontiguousarray(w.reshape(KC, 128, 256).transpose(1, 0, 2))


def _tile_mask(maskT_bf16):
    # [2048, 2048] -> [128, 16 m, 2048]
    return np.ascontiguousarray(
        maskT_bf16.reshape(MT, 128, S).transpose(1, 0, 2))


def kernel(q, k, v, mask, Wq, Wk, Wv, Wo, bo):
    global _BUILT
    if _BUILT is None:
        _BUILT = build_bass()
    nc = _BUILT

    BF = ml_dtypes.bfloat16
    q = np.asarray(q, np.float32)
    k = np.asarray(k, np.float32)
    v = np.asarray(v, np.float32)
    Wq = np.asarray(Wq, np.float32)
    Wk = np.asarray(Wk, np.float32)
    Wv = np.asarray(Wv, np.float32)
    Wo = np.asarray(Wo, np.float32)
    bo = np.asarray(bo, np.float32)
    mask = np.asarray(mask)

    cosT, sinT = _cos_sin_tables()
    perm = _rope_perm_cols()
    qTb = [_tile_xT(q[b].T.astype(BF)) for b in range(2)]
    kTb = [_tile_xT(k[b].T.astype(BF)) for b in range(2)]
    vTb = [_tile_vT(v[b].T.astype(BF)) for b in range(2)]
    maskTb = [_tile_mask(mask[b, 0].T.astype(BF)) for b in range(2)]

    in_maps = []
    for c in range(N_CORES):
        b = c // 4
        head_base = (c % 4) * 4
        cols = slice(head_base * 64, head_base * 64 + 256)
        in_maps.append({
            "qT": qTb[b], "kT": kTb[b], "vT": vTb[b],
            "wq": _tile_w(Wq[:, cols][:, perm].astype(BF)),
            "wk": _tile_w(Wk[:, cols][:, perm].astype(BF)),
            "wv": _tile_w(Wv[:, cols].astype(BF)),
            "wo": np.ascontiguousarray(
                Wo[cols, :].reshape(4, 64, DIM).transpose(1, 0, 2).astype(BF)),
            "cosT": cosT.astype(BF), "sinT": sinT.astype(BF),
            "maskT": maskTb[b],
        })

    kernel._last_in_maps = in_maps
    res = run_bass_kernel_spmd(nc, in_maps, core_ids=list(range(N_CORES)))
    out = np.zeros((2, S, DIM), np.float32)
    for c in range(N_CORES):
        out[c // 4] += res.results[c]["out_part"].astype(np.float32)
    out += bo[None, None, :]
    return out


# revision 24
# speedup vs baseline: 1.0384x; 1.0384x over previous
"""Multi-head attention (RoPE + mask + softmax) Trainium2 Bass kernel.

Sharding: 8 cores = 2 batches x 4 head-groups. Core c handles batch c//4,
local heads 4*(c%4) .. +4 (tensor-parallel on heads; Wq/Wk/Wv column-sharded,
Wo row-sharded; per-core partial outputs summed on host).

v3: all-bf16 matmul operands, f32 PSUM. Paired K=64 score matmuls (rows
0-63 / 64-127 concurrently). exp on ACT; mask-mul split DVE/gpsimd;
denominator: ones-column row of AV psum -> DVE reciprocal -> gpsimd
partition_broadcast -> fused DVE normalize. PSUM: 2x [128,1024] "work"
slots (scores/proj/outproj) + 2x [65,1024] "avp". Projections emit both
head-pair halves into one [128,1024] psum so RoPE runs 1024-wide.
v/q/k-proj + outproj interleaved into attention m-loops as PE filler;
mask chunks DMA'd just-in-time; output written straight from PSUM.
"""
import sys
sys.path.insert(0, '/opt/trn_rl_repo')
import math
import numpy as np
import ml_dtypes

import concourse.bass as bass
import concourse.mybir as mybir
import concourse.tile as tile
from concourse import bacc
from concourse.bass_utils import run_bass_kernel_spmd

F32 = mybir.dt.float32
BF16 = mybir.dt.bfloat16

S = 2048
DIM = 1024
HEAD_DIM = 64
N_CORES = 8
KC = DIM // 128          # 8 contraction chunks for projections
MT = S // 128            # 16 k-chunks in attention
QB = 1024                # q-block width
ROPE_THETA = 10000.0

_BUILT = None


def build_bass(dbg=False):
    nc = bacc.Bacc("TRN2", target_bir_lowering=False, debug=False)

    qT = nc.dram_tensor("qT", [4, 128, KC, 512], BF16, kind="ExternalInput").ap()
    kT = nc.dram_tensor("kT", [4, 128, KC, 512], BF16, kind="ExternalInput").ap()
    vT = nc.dram_tensor("vT", [MT, 128, KC, 128], BF16, kind="ExternalInput").ap()
    wq = nc.dram_tensor("wq", [128, KC, 256], BF16, kind="ExternalInput").ap()
    wk = nc.dram_tensor("wk", [128, KC, 256], BF16, kind="ExternalInput").ap()
    wv = nc.dram_tensor("wv", [128, KC, 256], BF16, kind="ExternalInput").ap()
    wo = nc.dram_tensor("wo", [64, 4, DIM], BF16, kind="ExternalInput").ap()
    cosT = nc.dram_tensor("cosT", [128, S], BF16, kind="ExternalInput").ap()
    sinT = nc.dram_tensor("sinT", [128, S], BF16, kind="ExternalInput").ap()
    maskT = nc.dram_tensor("maskT", [128, MT, S], BF16, kind="ExternalInput").ap()
    out_part = nc.dram_tensor("out_part", [S, DIM], BF16, kind="ExternalOutput").ap()
    if dbg:
        qhT_d = nc.dram_tensor("qhT_d", [128, 2, S], BF16,
                               kind="ExternalOutput").ap()
        khT_d = nc.dram_tensor("khT_d", [128, 2, S], BF16,
                               kind="ExternalOutput").ap()
        vh_d = nc.dram_tensor("vh_d", [128, MT, 4 * 65], BF16,
                              kind="ExternalOutput").ap()
        at_d = nc.dram_tensor("at_d", [128, 8, QB], BF16,
                              kind="ExternalOutput").ap()
        outT_d = nc.dram_tensor("outT_d", [64, 4, S], BF16,
                                kind="ExternalOutput").ap()

    with tile.TileContext(nc) as tc:
        with tc.tile_pool(name="persist", bufs=1) as persist, \
             tc.tile_pool(name="ps", bufs=2, space="PSUM") as ps, \
             tc.tile_pool(name="xts", bufs=4) as xts, \
             tc.tile_pool(name="vx", bufs=4) as vxp, \
             tc.tile_pool(name="rope", bufs=2) as rope, \
             tc.tile_pool(name="attn", bufs=3) as attnp, \
             tc.tile_pool(name="atmp", bufs=8) as atmp, \
             tc.tile_pool(name="dn", bufs=2) as dnp, \
             tc.tile_pool(name="outp", bufs=2) as outp:

            qhT = persist.tile([128, 2, S], BF16)     # [chunk-part, hp, s]
            khT = persist.tile([128, 2, S], BF16)
            vh = persist.tile([128, MT, 4 * 65], BF16)
            outT = persist.tile([64, 4, S], BF16)
            wo_sb = persist.tile([64, 4, DIM], BF16)
            wq_sb = persist.tile([128, KC, 256], BF16)
            wk_sb = persist.tile([128, KC, 256], BF16)
            wv_sb = persist.tile([128, KC, 256], BF16)
            cos_sb = persist.tile([128, 2, S], BF16)  # duplicated per head-pair
            sin_sb = persist.tile([128, 2, S], BF16)
            mk = persist.tile([128, MT, QB], BF16)

            # ---- weight DMAs first (sync), tables + early mask (scalar) ----
            nc.sync.dma_start(out=wk_sb, in_=wk)
            nc.sync.dma_start(out=wq_sb, in_=wq)
            nc.sync.dma_start(out=wv_sb, in_=wv)
            for hp in range(2):
                nc.scalar.dma_start(out=cos_sb[:, hp, :], in_=cosT)
                nc.scalar.dma_start(out=sin_sb[:, hp, :], in_=sinT)
            for m in range(4):
                nc.scalar.dma_start(out=mk[:, m, :], in_=maskT[:, m, 0:QB])
            # ones column for the denominator rows of vh
            nc.vector.memset(
                vh.rearrange("p m (h x) -> p m h x", x=65)[:, :, :, 64:65], 1.0)

            _xcache = {}

            def proj_rope(xdram, w_sb, dstT, sblk):
                """One sblk projection (both head-pairs) with fused RoPE."""
                key = (id(xdram), sblk)
                if key not in _xcache:
                    halves = []
                    for kh2 in range(2):
                        xh = xts.tile([128, 4, 512], BF16, tag="xts",
                                      name=f"x_{id(xdram) & 0xffff}_{sblk}_{kh2}")
                        nc.sync.dma_start(out=xh, in_=xdram[sblk, :, kh2 * 4:(kh2 + 1) * 4, :])
                        halves.append(xh)
                    _xcache[key] = halves
                halves = _xcache[key]
                ss = slice(sblk * 512, (sblk + 1) * 512)
                psum = ps.tile([128, 2, 512], F32, tag="work",
                               name=f"pj_{id(xdram) & 0xffff}_{sblk}")
                for hp in range(2):
                    for kc in range(KC):
                        nc.tensor.matmul(
                            psum[:, hp, :],
                            lhsT=w_sb[:, kc, hp * 128:(hp + 1) * 128],
                            rhs=halves[kc // 4][:, kc % 4, :],
                            start=(kc == 0), stop=(kc == KC - 1))
                t = rope.tile([128, 2, 512], BF16, tag="t")
                u = rope.tile([128, 2, 512], BF16, tag="u")
                nc.vector.tensor_mul(t, psum, cos_sb[:, :, ss])
                nc.vector.tensor_mul(u, psum, sin_sb[:, :, ss])
                us = rope.tile([128, 2, 512], BF16, tag="us")
                for blk in range(4):
                    a, b2 = blk * 32, (blk ^ 1) * 32
                    nc.sync.dma_start(out=us[a:a + 32, :, :], in_=u[b2:b2 + 32, :, :])
                nc.vector.tensor_add(dstT[:, :, ss], t, us)

            def proj_rope_hp(xdram, w_sb, dstT, sblk, hp):
                """Half projection unit (one head-pair) for finer fillers."""
                key = (id(xdram), sblk)
                if key not in _xcache:
                    halves = []
                    for kh2 in range(2):
                        xh = xts.tile([128, 4, 512], BF16, tag="xts",
                                      name=f"xh_{id(xdram) & 0xffff}_{sblk}_{kh2}")
                        nc.sync.dma_start(out=xh,
                                          in_=xdram[sblk, :, kh2 * 4:(kh2 + 1) * 4, :])
                        halves.append(xh)
                    _xcache[key] = halves
                halves = _xcache[key]
                ss = slice(sblk * 512, (sblk + 1) * 512)
                psum = ps.tile([128, 512], F32, tag="work",
                               name=f"pjh_{id(xdram) & 0xffff}_{sblk}_{hp}")
                for kc in range(KC):
                    nc.tensor.matmul(
                        psum,
                        lhsT=w_sb[:, kc, hp * 128:(hp + 1) * 128],
                        rhs=halves[kc // 4][:, kc % 4, :],
                        start=(kc == 0), stop=(kc == KC - 1))
                t = rope.tile([128, 512], BF16, tag="t",
                              name=f"th_{id(xdram) & 0xffff}_{sblk}_{hp}")
                u = rope.tile([128, 512], BF16, tag="u",
                              name=f"uh_{id(xdram) & 0xffff}_{sblk}_{hp}")
                nc.vector.tensor_mul(t, psum, cos_sb[:, hp, ss])
                nc.vector.tensor_mul(u, psum, sin_sb[:, hp, ss])
                us = rope.tile([128, 512], BF16, tag="us",
                               name=f"ush_{id(xdram) & 0xffff}_{sblk}_{hp}")
                for blk in range(4):
                    a, b2 = blk * 32, (blk ^ 1) * 32
                    nc.sync.dma_start(out=us[a:a + 32, :], in_=u[b2:b2 + 32, :])
                nc.vector.tensor_add(dstT[:, hp, ss], t, us)

            def vproj(sc):
                v_sb = vxp.tile([128, KC, 128], BF16, tag="vts", name=f"v_{sc}")
                nc.sync.dma_start(out=v_sb, in_=vT[sc])
                psum = ps.tile([128, 256], F32, tag="work", name=f"vp_{sc}")
                for kc in range(KC):
                    nc.tensor.matmul(
                        psum, lhsT=v_sb[:, kc, :], rhs=wv_sb[:, kc, :],
                        start=(kc == 0), stop=(kc == KC - 1))
                nc.vector.tensor_copy(
                    vh[:, sc, :].rearrange("p (h x) -> p h x", x=65)[:, :, 0:64],
                    psum.rearrange("p (h x) -> p h x", x=64))

            def outproj(sc):
                """Output projection for one 128-row s-chunk, PSUM -> DRAM."""
                wps = ps.tile([128, 2, 512], F32, tag="work", name=f"op_{sc}")
                for nb in range(2):
                    for h in range(4):
                        nc.tensor.matmul(
                            wps[:, nb, :],
                            lhsT=outT[0:64, h, sc * 128:(sc + 1) * 128],
                            rhs=wo_sb[0:64, h, nb * 512:(nb + 1) * 512],
                            start=(h == 0), stop=(h == 3))
                oc = outp.tile([128, DIM], BF16, tag="oc", name=f"oc_{sc}")
                nc.scalar.copy(oc, wps.rearrange("p a b -> p (a b)"))
                nc.sync.dma_start(
                    out=out_part[sc * 128:(sc + 1) * 128, :], in_=oc)

            def attention_unit(qb, hp, fillers, dump=None):
                """Attention for (q-block, head-pair): 16 m-chunks + normalize.

                fillers: dict m -> list of callables emitted at the top of
                that m-iteration (PE filler work + JIT DMAs).
                """
                qs = slice(qb * QB, (qb + 1) * QB)
                avp = [ps.tile([65, QB], F32, tag="avp", name=f"avp{qb}_{hp}_{i}")
                       for i in range(2)]
                atm_q = []

                def av_pair(mm):
                    atm2 = atm_q.pop(0)
                    for h2 in range(2):
                        h = 2 * hp + h2
                        for q2 in range(2):
                            nc.tensor.matmul(
                                avp[h2][:, q2 * 512:(q2 + 1) * 512],
                                lhsT=vh[:, mm, h * 65:(h + 1) * 65],
                                rhs=atm2[h2][:, q2 * 512:(q2 + 1) * 512],
                                start=(mm == 0), stop=(mm == MT - 1))

                for m in range(MT):
                    for f in fillers.get(m, ()):
                        if getattr(f, "is_dma", False):
                            f()
                    sps_t = [None, None]
                    for h2 in range(2):
                        sps_t[h2] = ps.tile([128, QB], F32, tag="work",
                                            name=f"sps_{qb}_{hp}_{m}_{h2}")
                    for q2 in range(2):
                        for h2 in range(2):
                            hb = slice(h2 * 64, (h2 + 1) * 64)
                            nc.tensor.matmul(
                                sps_t[h2][:, q2 * 512:(q2 + 1) * 512],
                                lhsT=khT[hb, hp, m * 128:(m + 1) * 128],
                                rhs=qhT[hb, hp, qs][:, q2 * 512:(q2 + 1) * 512],
                                start=True, stop=True)
                    atm2 = [None, None]
                    for h2 in range(2):
                        at = attnp.tile([128, QB], BF16, tag="at")
                        nc.scalar.activation(
                            at, sps_t[h2], mybir.ActivationFunctionType.Exp,
                            scale=1.0 / math.sqrt(HEAD_DIM))
                        atm2[h2] = atmp.tile([128, QB], BF16, tag="atm",
                                              name=f"atm_{qb}_{hp}_{m}_{h2}")
                        eng = nc.gpsimd if (2 * m + h2) % 4 == 3 else nc.vector
                        eng.tensor_mul(atm2[h2], at, mk[:, m, :])
                        if dump is not None and m < 4:
                            nc.sync.dma_start(out=dump[:, 2 * m + h2, :],
                                              in_=atm2[h2])
                    atm_q.append(atm2)
                    for f in fillers.get(m, ()):
                        if not getattr(f, "is_dma", False):
                            f()
                    if m >= 3:
                        av_pair(m - 3)
                av_pair(MT - 3)
                av_pair(MT - 2)
                av_pair(MT - 1)
                # normalize: recip of denominator row, broadcast, fused evict
                rdnb = [None, None]
                for h2 in range(2):
                    row = dnp.tile([1, QB], F32, tag="row",
                                   name=f"row{qb}_{hp}_{h2}")
                    nc.vector.tensor_copy(row, avp[h2][64:65, :])
                    rf = dnp.tile([1, QB], F32, tag="rf",
                                  name=f"rf{qb}_{hp}_{h2}")
                    nc.vector.reciprocal_approx_fast(rf, row)
                    rdnb[h2] = dnp.tile([64, QB], F32, tag="rdnb",
                                        name=f"rdnb{qb}_{hp}_{h2}")
                    nc.gpsimd.partition_broadcast(rdnb[h2], rf)
                for h2 in range(2):
                    h = 2 * hp + h2
                    nc.vector.tensor_mul(outT[0:64, h, qs], avp[h2][0:64, :],
                                         rdnb[h2])

            # ---------------- emission schedule ----------------
            for sblk in (0, 1):
                proj_rope(kT, wk_sb, khT, sblk)
            for sblk in (0, 1):
                proj_rope(qT, wq_sb, qhT, sblk)

            # qb0-hp0: fill with k sblk2/3, all v, JIT mask DMAs
            f0 = {m: [] for m in range(MT)}
            f0[0].append(lambda: vproj(0))
            f0[0].append(lambda: vproj(1))
            f0[1].append(lambda: proj_rope(kT, wk_sb, khT, 2))
            f0[3].append(lambda: proj_rope(kT, wk_sb, khT, 3))
            for m in range(MT):
                if m + 2 < MT:
                    f0[m].append(lambda sc=m + 2: vproj(sc))
                if m + 4 < MT:
                    _d = lambda mm=m + 4: nc.scalar.dma_start(
                        out=mk[:, mm, :], in_=maskT[:, mm, 0:QB])
                    _d.is_dma = True
                    f0[m].append(_d)
            attention_unit(0, 0, f0, dump=at_d if dbg else None)

            # qb0-hp1: fill with q sblk2/3, wo DMA
            f1 = {m: [] for m in range(MT)}
            _dwo = lambda: nc.scalar.dma_start(out=wo_sb, in_=wo)
            _dwo.is_dma = True
            f1[0].append(_dwo)
            f1[2].append(lambda: proj_rope(qT, wq_sb, qhT, 2))
            f1[6].append(lambda: proj_rope(qT, wq_sb, qhT, 3))
            for i, m in enumerate((12, 13, 14, 15)):
                _d1 = lambda mm=i: nc.scalar.dma_start(
                    out=mk[:, mm, :], in_=maskT[:, mm, QB:S])
                _d1.is_dma = True
                f1[m].append(_d1)
            attention_unit(0, 1, f1)

            # qb1: fill with outproj of qb0 rows
            f2 = {m: [] for m in range(MT)}
            for i, m in enumerate((1, 3, 5, 7, 9, 11, 13, 15)):
                f2[m].append(lambda sc=i: outproj(sc))
            for m in range(MT):
                if m + 4 < MT:
                    _d2 = lambda mm=m + 4: nc.scalar.dma_start(
                        out=mk[:, mm, :], in_=maskT[:, mm, QB:S])
                    _d2.is_dma = True
                    f2[m].append(_d2)
            attention_unit(1, 0, f2)
            f3 = {m: [] for m in range(MT)}

            attention_unit(1, 1, f3)
            for sc in range(8, MT):
                outproj(sc)

            if dbg:
                nc.sync.dma_start(out=qhT_d, in_=qhT)
                nc.sync.dma_start(out=khT_d, in_=khT)
                nc.sync.dma_start(out=vh_d, in_=vh)
                nc.sync.dma_start(out=outT_d, in_=outT)

    nc.compile()
    return nc


def _rope_perm_cols():
    """Column permutation of the 256-wide W slice for one core's 4 heads.

    Chunk c (0,1) holds local heads 2c, 2c+1 as rows
    [hA_even(32) | hA_odd(32) | hB_even(32) | hB_odd(32)].
    """
    cols = []
    for c in range(2):
        for j2 in range(2):          # which head within the chunk
            head = 2 * c + j2
            for blk in range(2):     # 0: even dims, 1: odd dims
                for i in range(32):
                    cols.append(head * 64 + 2 * i + blk)
    return np.array(cols)


def _cos_sin_tables():
    inv_freq = 1.0 / (ROPE_THETA ** (np.arange(0, HEAD_DIM, 2, dtype=np.float64)
                                     / HEAD_DIM))          # [32]
    ang = np.arange(S, dtype=np.float64)[None, :] * inv_freq[:, None]  # [32, S]
    cos32 = np.cos(ang).astype(np.float32)
    sin32 = np.sin(ang).astype(np.float32)
    cosT = np.tile(cos32, (4, 1))                           # [128, S]
    # sign: +sin at even-dim rows (blocks 0, 2), -sin at odd-dim rows (1, 3)
    sinT = np.concatenate([sin32, -sin32, sin32, -sin32], axis=0)
    return np.ascontiguousarray(cosT), np.ascontiguousarray(sinT)


def _tile_xT(xT):
    # [1024, 2048] -> [4 sblk, 128 part, 8 kc, 512]
    return np.ascontiguousarray(
        xT.reshape(KC, 128, 4, 512).transpose(2, 1, 0, 3))


def _tile_vT(vT):
    # [1024, 2048] -> [16 sc, 128 part, 8 kc, 128]
    return np.ascontiguousarray(
        vT.reshape(KC, 128, MT, 128).transpose(2, 1, 0, 3))


def _tile_w(w):
    # [1024, 256] -> [128, 8, 256]
    return np.ascontiguousarray(w.reshape(KC, 128, 256).transpose(1, 0, 2))


def _tile_mask(maskT_bf16):
    # [2048, 2048] -> [128, 16 m, 2048]
    return np.ascontiguousarray(
        maskT_bf16.reshape(MT, 128, S).transpose(1, 0, 2))


def kernel(q, k, v, mask, Wq, Wk, Wv, Wo, bo):
    global _BUILT
    if _BUILT is None:
        _BUILT = build_bass()
    nc = _BUILT

    BF = ml_dtypes.bfloat16
    q = np.asarray(q, np.float32)
    k = np.asarray(k, np.float32)
    v = np.asarray(v, np.float32)
    Wq = np.asarray(Wq, np.float32)
    Wk = np.asarray(Wk, np.float32)
    Wv = np.asarray(Wv, np.float32)
    Wo = np.asarray(Wo, np.float32)
    bo = np.asarray(bo, np.float32)
    mask = np.asarray(mask)

    cosT, sinT = _cos_sin_tables()
    perm = _rope_perm_cols()
    qTb = [_tile_xT(q[b].T.astype(BF)) for b in range(2)]
    kTb = [_tile_xT(k[b].T.astype(BF)) for b in range(2)]
    vTb = [_tile_vT(v[b].T.astype(BF)) for b in range(2)]
    maskTb = [_tile_mask(mask[b, 0].T.astype(BF)) for b in range(2)]

    in_maps = []
    for c in range(N_CORES):
        b = c // 4
        head_base = (c % 4) * 4
        cols = slice(head_base * 64, head_base * 64 + 256)
        in_maps.append({
            "qT": qTb[b], "kT": kTb[b], "vT": vTb[b],
            "wq": _tile_w(Wq[:, cols][:, perm].astype(BF)),
            "wk": _tile_w(Wk[:, cols][:, perm].astype(BF)),
            "wv": _tile_w(Wv[:, cols].astype(BF)),
            "wo": np.ascontiguousarray(
                Wo[cols, :].reshape(4, 64, DIM).transpose(1, 0, 2).astype(BF)),
            "cosT": cosT.astype(BF), "sinT": sinT.astype(BF),
            "maskT": maskTb[b],
        })

    kernel._last_in_maps = in_maps
    res = run_bass_kernel_spmd(nc, in_maps, core_ids=list(range(N_CORES)))
    out = np.zeros((2, S, DIM), np.float32)
    for c in range(N_CORES):
        out[c // 4] += res.results[c]["out_part"].astype(np.float32)
    out += bo[None, None, :]
    return out


# revision 25
# speedup vs baseline: 1.1019x; 1.0612x over previous
"""Multi-head attention (RoPE + mask + softmax) Trainium2 Bass kernel.

Sharding: 8 cores = 2 batches x 4 head-groups. Core c handles batch c//4,
local heads 4*(c%4) .. +4 (tensor-parallel on heads; Wq/Wk/Wv column-sharded,
Wo row-sharded; per-core partial outputs summed on host).

v3: all-bf16 matmul operands, f32 PSUM. Paired K=64 score matmuls (rows
0-63 / 64-127 concurrently). exp on ACT; mask-mul split DVE/gpsimd;
denominator: ones-column row of AV psum -> DVE reciprocal -> gpsimd
partition_broadcast -> fused DVE normalize. PSUM: 2x [128,1024] "work"
slots (scores/proj/outproj) + 2x [65,1024] "avp". Projections emit both
head-pair halves into one [128,1024] psum so RoPE runs 1024-wide.
v/q/k-proj + outproj interleaved into attention m-loops as PE filler;
mask chunks DMA'd just-in-time; output written straight from PSUM.
"""
import sys
sys.path.insert(0, '/opt/trn_rl_repo')
import math
import numpy as np
import ml_dtypes

import concourse.bass as bass
import concourse.mybir as mybir
import concourse.tile as tile
from concourse import bacc
from concourse.bass_utils import run_bass_kernel_spmd

F32 = mybir.dt.float32
BF16 = mybir.dt.bfloat16

S = 2048
DIM = 1024
HEAD_DIM = 64
N_CORES = 8
KC = DIM // 128          # 8 contraction chunks for projections
MT = S // 128            # 16 k-chunks in attention
QB = 1024                # q-block width
ROPE_THETA = 10000.0

_BUILT = None


def build_bass(dbg=False):
    nc = bacc.Bacc("TRN2", target_bir_lowering=False, debug=False)

    qT = nc.dram_tensor("qT", [4, 128, KC, 512], BF16, kind="ExternalInput").ap()
    kT = nc.dram_tensor("kT", [4, 128, KC, 512], BF16, kind="ExternalInput").ap()
    vT = nc.dram_tensor("vT", [MT, 128, KC, 128], BF16, kind="ExternalInput").ap()
    wq = nc.dram_tensor("wq", [128, KC, 256], BF16, kind="ExternalInput").ap()
    wk = nc.dram_tensor("wk", [128, KC, 256], BF16, kind="ExternalInput").ap()
    wv = nc.dram_tensor("wv", [128, KC, 256], BF16, kind="ExternalInput").ap()
    wo = nc.dram_tensor("wo", [64, 4, DIM], BF16, kind="ExternalInput").ap()
    cosT = nc.dram_tensor("cosT", [128, S], BF16, kind="ExternalInput").ap()
    sinT = nc.dram_tensor("sinT", [128, S], BF16, kind="ExternalInput").ap()
    maskT = nc.dram_tensor("maskT", [128, MT, S], BF16, kind="ExternalInput").ap()
    out_part = nc.dram_tensor("out_part", [S, DIM], BF16, kind="ExternalOutput").ap()
    if dbg:
        qhT_d = nc.dram_tensor("qhT_d", [128, 2, S], BF16,
                               kind="ExternalOutput").ap()
        khT_d = nc.dram_tensor("khT_d", [128, 2, S], BF16,
                               kind="ExternalOutput").ap()
        vh_d = nc.dram_tensor("vh_d", [128, MT, 4 * 65], BF16,
                              kind="ExternalOutput").ap()
        at_d = nc.dram_tensor("at_d", [128, 8, QB], BF16,
                              kind="ExternalOutput").ap()
        outT_d = nc.dram_tensor("outT_d", [64, 4, S], BF16,
                                kind="ExternalOutput").ap()

    with tile.TileContext(nc) as tc:
        with tc.tile_pool(name="persist", bufs=1) as persist, \
             tc.tile_pool(name="ps", bufs=2, space="PSUM") as ps, \
             tc.tile_pool(name="xts", bufs=4) as xts, \
             tc.tile_pool(name="vx", bufs=4) as vxp, \
             tc.tile_pool(name="rope", bufs=2) as rope, \
             tc.tile_pool(name="attn", bufs=3) as attnp, \
             tc.tile_pool(name="atmp", bufs=8) as atmp, \
             tc.tile_pool(name="dn", bufs=2) as dnp, \
             tc.tile_pool(name="outp", bufs=2) as outp:

            qhT = persist.tile([128, 2, S], BF16)     # [chunk-part, hp, s]
            khT = persist.tile([128, 2, S], BF16)
            vh = persist.tile([128, MT, 4 * 65], BF16)
            outT = persist.tile([64, 4, S], BF16)
            wo_sb = persist.tile([64, 4, DIM], BF16)
            wq_sb = persist.tile([128, KC, 256], BF16)
            wk_sb = persist.tile([128, KC, 256], BF16)
            wv_sb = persist.tile([128, KC, 256], BF16)
            cos_sb = persist.tile([128, 2, S], BF16)  # duplicated per head-pair
            sin_sb = persist.tile([128, 2, S], BF16)
            mk = persist.tile([128, MT, QB], BF16)

            # ---- weight DMAs first (sync), tables + early mask (scalar) ----
            nc.sync.dma_start(out=wk_sb, in_=wk)
            nc.sync.dma_start(out=wq_sb, in_=wq)
            nc.sync.dma_start(out=wv_sb, in_=wv)
            for hp in range(2):
                nc.scalar.dma_start(out=cos_sb[:, hp, :], in_=cosT)
                nc.scalar.dma_start(out=sin_sb[:, hp, :], in_=sinT)
            for m in range(4):
                nc.scalar.dma_start(out=mk[:, m, :], in_=maskT[:, m, 0:QB])
            # ones column for the denominator rows of vh
            nc.vector.memset(
                vh.rearrange("p m (h x) -> p m h x", x=65)[:, :, :, 64:65], 1.0)

            _xcache = {}

            def proj_rope(xdram, w_sb, dstT, sblk):
                """One sblk projection (both head-pairs) with fused RoPE."""
                key = (id(xdram), sblk)
                if key not in _xcache:
                    halves = []
                    for kh2 in range(2):
                        xh = xts.tile([128, 4, 512], BF16, tag="xts",
                                      name=f"x_{id(xdram) & 0xffff}_{sblk}_{kh2}")
                        nc.sync.dma_start(out=xh, in_=xdram[sblk, :, kh2 * 4:(kh2 + 1) * 4, :])
                        halves.append(xh)
                    _xcache[key] = halves
                halves = _xcache[key]
                ss = slice(sblk * 512, (sblk + 1) * 512)
                psum = ps.tile([128, 2, 512], F32, tag="work",
                               name=f"pj_{id(xdram) & 0xffff}_{sblk}")
                for hp in range(2):
                    for kc in range(KC):
                        nc.tensor.matmul(
                            psum[:, hp, :],
                            lhsT=w_sb[:, kc, hp * 128:(hp + 1) * 128],
                            rhs=halves[kc // 4][:, kc % 4, :],
                            start=(kc == 0), stop=(kc == KC - 1))
                t = rope.tile([128, 2, 512], BF16, tag="t")
                u = rope.tile([128, 2, 512], BF16, tag="u")
                nc.vector.tensor_mul(t, psum, cos_sb[:, :, ss])
                nc.vector.tensor_mul(u, psum, sin_sb[:, :, ss])
                us = rope.tile([128, 2, 512], BF16, tag="us")
                for blk in range(4):
                    a, b2 = blk * 32, (blk ^ 1) * 32
                    nc.sync.dma_start(out=us[a:a + 32, :, :], in_=u[b2:b2 + 32, :, :])
                nc.vector.tensor_add(dstT[:, :, ss], t, us)

            def proj_rope_hp(xdram, w_sb, dstT, sblk, hp):
                """Half projection unit (one head-pair) for finer fillers."""
                key = (id(xdram), sblk)
                if key not in _xcache:
                    halves = []
                    for kh2 in range(2):
                        xh = xts.tile([128, 4, 512], BF16, tag="xts",
                                      name=f"xh_{id(xdram) & 0xffff}_{sblk}_{kh2}")
                        nc.sync.dma_start(out=xh,
                                          in_=xdram[sblk, :, kh2 * 4:(kh2 + 1) * 4, :])
                        halves.append(xh)
                    _xcache[key] = halves
                halves = _xcache[key]
                ss = slice(sblk * 512, (sblk + 1) * 512)
                psum = ps.tile([128, 512], F32, tag="work",
                               name=f"pjh_{id(xdram) & 0xffff}_{sblk}_{hp}")
                for kc in range(KC):
                    nc.tensor.matmul(
                        psum,
                        lhsT=w_sb[:, kc, hp * 128:(hp + 1) * 128],
                        rhs=halves[kc // 4][:, kc % 4, :],
                        start=(kc == 0), stop=(kc == KC - 1))
                t = rope.tile([128, 512], BF16, tag="t",
                              name=f"th_{id(xdram) & 0xffff}_{sblk}_{hp}")
                u = rope.tile([128, 512], BF16, tag="u",
                              name=f"uh_{id(xdram) & 0xffff}_{sblk}_{hp}")
                nc.vector.tensor_mul(t, psum, cos_sb[:, hp, ss])
                nc.vector.tensor_mul(u, psum, sin_sb[:, hp, ss])
                us = rope.tile([128, 512], BF16, tag="us",
                               name=f"ush_{id(xdram) & 0xffff}_{sblk}_{hp}")
                for blk in range(4):
                    a, b2 = blk * 32, (blk ^ 1) * 32
                    nc.sync.dma_start(out=us[a:a + 32, :], in_=u[b2:b2 + 32, :])
                nc.vector.tensor_add(dstT[:, hp, ss], t, us)

            def vproj(sc):
                v_sb = vxp.tile([128, KC, 128], BF16, tag="vts", name=f"v_{sc}")
                nc.sync.dma_start(out=v_sb, in_=vT[sc])
                psum = ps.tile([128, 256], F32, tag="work", name=f"vp_{sc}")
                for kc in range(KC):
                    nc.tensor.matmul(
                        psum, lhsT=v_sb[:, kc, :], rhs=wv_sb[:, kc, :],
                        start=(kc == 0), stop=(kc == KC - 1))
                nc.vector.tensor_copy(
                    vh[:, sc, :].rearrange("p (h x) -> p h x", x=65)[:, :, 0:64],
                    psum.rearrange("p (h x) -> p h x", x=64))

            def outproj(sc):
                """Output projection for one 128-row s-chunk, PSUM -> DRAM."""
                wps = ps.tile([128, 2, 512], F32, tag="work", name=f"op_{sc}")
                for nb in range(2):
                    for h in range(4):
                        nc.tensor.matmul(
                            wps[:, nb, :],
                            lhsT=outT[0:64, h, sc * 128:(sc + 1) * 128],
                            rhs=wo_sb[0:64, h, nb * 512:(nb + 1) * 512],
                            start=(h == 0), stop=(h == 3))
                oc = outp.tile([128, DIM], BF16, tag="oc", name=f"oc_{sc}")
                nc.scalar.copy(oc, wps.rearrange("p a b -> p (a b)"))
                nc.sync.dma_start(
                    out=out_part[sc * 128:(sc + 1) * 128, :], in_=oc)

            def attention_unit(qb, hp, fillers, dump=None):
                """Attention for (q-block, head-pair): 16 m-chunks + normalize.

                fillers: dict m -> list of callables emitted at the top of
                that m-iteration (PE filler work + JIT DMAs).
                """
                qs = slice(qb * QB, (qb + 1) * QB)
                avp = [ps.tile([65, QB], F32, tag="avp", name=f"avp{qb}_{hp}_{i}")
                       for i in range(2)]
                atm_q = []

                def av_pair(mm):
                    atm2 = atm_q.pop(0)
                    for h2 in range(2):
                        h = 2 * hp + h2
                        for q2 in range(2):
                            nc.tensor.matmul(
                                avp[h2][:, q2 * 512:(q2 + 1) * 512],
                                lhsT=vh[:, mm, h * 65:(h + 1) * 65],
                                rhs=atm2[h2][:, q2 * 512:(q2 + 1) * 512],
                                start=(mm == 0), stop=(mm == MT - 1))

                for m in range(MT):
                    for f in fillers.get(m, ()):
                        if getattr(f, "is_dma", False):
                            f()
                    sps_t = [None, None]
                    for h2 in range(2):
                        sps_t[h2] = ps.tile([128, QB], F32, tag="work",
                                            name=f"sps_{qb}_{hp}_{m}_{h2}")
                    for q2 in range(2):
                        for h2 in range(2):
                            hb = slice(h2 * 64, (h2 + 1) * 64)
                            nc.tensor.matmul(
                                sps_t[h2][:, q2 * 512:(q2 + 1) * 512],
                                lhsT=khT[hb, hp, m * 128:(m + 1) * 128],
                                rhs=qhT[hb, hp, qs][:, q2 * 512:(q2 + 1) * 512],
                                start=True, stop=True)
                    atm2 = [None, None]
                    for h2 in range(2):
                        at = attnp.tile([128, QB], BF16, tag="at")
                        nc.scalar.activation(
                            at, sps_t[h2], mybir.ActivationFunctionType.Exp,
                            scale=1.0 / math.sqrt(HEAD_DIM))
                        atm2[h2] = atmp.tile([128, QB], BF16, tag="atm",
                                              name=f"atm_{qb}_{hp}_{m}_{h2}")
                        eng = nc.gpsimd if (2 * m + h2) % 4 == 3 else nc.vector
                        eng.tensor_mul(atm2[h2], at, mk[:, m, :])
                        if dump is not None and m < 4:
                            nc.sync.dma_start(out=dump[:, 2 * m + h2, :],
                                              in_=atm2[h2])
                    atm_q.append(atm2)
                    for f in fillers.get(m, ()):
                        if not getattr(f, "is_dma", False):
                            f()
                    if m >= 3:
                        av_pair(m - 3)
                av_pair(MT - 3)
                av_pair(MT - 2)
                av_pair(MT - 1)
                # normalize: recip of denominator row, broadcast, fused evict
                rdnb = [None, None]
                for h2 in range(2):
                    row = dnp.tile([1, QB], F32, tag="row",
                                   name=f"row{qb}_{hp}_{h2}")
                    nc.vector.tensor_copy(row, avp[h2][64:65, :])
                    rf = dnp.tile([1, QB], F32, tag="rf",
                                  name=f"rf{qb}_{hp}_{h2}")
                    nc.vector.reciprocal_approx_fast(rf, row)
                    rdnb[h2] = dnp.tile([64, QB], F32, tag="rdnb",
                                        name=f"rdnb{qb}_{hp}_{h2}")
                    nc.gpsimd.partition_broadcast(rdnb[h2], rf)
                for h2 in range(2):
                    h = 2 * hp + h2
                    nc.vector.tensor_mul(outT[0:64, h, qs], avp[h2][0:64, :],
                                         rdnb[h2])

            # ---------------- emission schedule ----------------
            proj_rope(kT, wk_sb, khT, 0)
            proj_rope(qT, wq_sb, qhT, 0)
            proj_rope(qT, wq_sb, qhT, 1)
            proj_rope(kT, wk_sb, khT, 1)

            # qb0-hp0: fill with k sblk2/3, all v, JIT mask DMAs
            f0 = {m: [] for m in range(MT)}
            f0[0].append(lambda: vproj(0))
            f0[0].append(lambda: vproj(1))
            f0[1].append(lambda: proj_rope(kT, wk_sb, khT, 2))
            f0[3].append(lambda: proj_rope(kT, wk_sb, khT, 3))
            for m in range(MT):
                if m + 2 < MT:
                    f0[m].append(lambda sc=m + 2: vproj(sc))
                if m + 4 < MT:
                    _d = lambda mm=m + 4: nc.scalar.dma_start(
                        out=mk[:, mm, :], in_=maskT[:, mm, 0:QB])
                    _d.is_dma = True
                    f0[m].append(_d)
            attention_unit(0, 0, f0, dump=at_d if dbg else None)

            # qb0-hp1: fill with q sblk2/3, wo DMA
            f1 = {m: [] for m in range(MT)}
            _dwo = lambda: nc.scalar.dma_start(out=wo_sb, in_=wo)
            _dwo.is_dma = True
            f1[0].append(_dwo)
            f1[2].append(lambda: proj_rope(qT, wq_sb, qhT, 2))
            f1[6].append(lambda: proj_rope(qT, wq_sb, qhT, 3))
            for i, m in enumerate((12, 13, 14, 15)):
                _d1 = lambda mm=i: nc.scalar.dma_start(
                    out=mk[:, mm, :], in_=maskT[:, mm, QB:S])
                _d1.is_dma = True
                f1[m].append(_d1)
            attention_unit(0, 1, f1)

            # qb1: fill with outproj of qb0 rows
            f2 = {m: [] for m in range(MT)}
            for i, m in enumerate((1, 3, 5, 7, 9, 11)):
                f2[m].append(lambda sc=i: outproj(sc))
            for m in range(MT):
                if m + 4 < MT:
                    _d2 = lambda mm=m + 4: nc.scalar.dma_start(
                        out=mk[:, mm, :], in_=maskT[:, mm, QB:S])
                    _d2.is_dma = True
                    f2[m].append(_d2)
            attention_unit(1, 0, f2)
            f3 = {m: [] for m in range(MT)}
            for i, m in enumerate((1, 5)):
                f3[m].append(lambda sc=6 + i: outproj(sc))
            attention_unit(1, 1, f3)
            for sc in range(8, MT):
                outproj(sc)

            if dbg:
                nc.sync.dma_start(out=qhT_d, in_=qhT)
                nc.sync.dma_start(out=khT_d, in_=khT)
                nc.sync.dma_start(out=vh_d, in_=vh)
                nc.sync.dma_start(out=outT_d, in_=outT)

    nc.compile()
    return nc


def _rope_perm_cols():
    """Column permutation of the 256-wide W slice for one core's 4 heads.

    Chunk c (0,1) holds local heads 2c, 2c+1 as rows
    [hA_even(32) | hA_odd(32) | hB_even(32) | hB_odd(32)].
    """
    cols = []
    for c in range(2):
        for j2 in range(2):          # which head within the chunk
            head = 2 * c + j2
            for blk in range(2):     # 0: even dims, 1: odd dims
                for i in range(32):
                    cols.append(head * 64 + 2 * i + blk)
    return np.array(cols)


def _cos_sin_tables():
    inv_freq = 1.0 / (ROPE_THETA ** (np.arange(0, HEAD_DIM, 2, dtype=np.float64)
                                     / HEAD_DIM))          # [32]
    ang = np.arange(S, dtype=np.float64)[None, :] * inv_freq[:, None]  # [32, S]
    cos32 = np.cos(ang).astype(np.float32)
    sin32 = np.sin(ang).astype(np.float32)
    cosT = np.tile(cos32, (4, 1))                           # [128, S]
    # sign: +sin at even-dim rows (blocks 0, 2), -sin at odd-dim rows (1, 3)
    sinT = np.concatenate([sin32, -sin32, sin32, -sin32], axis=0)
    return np.ascontiguousarray(cosT), np.ascontiguousarray(sinT)


def _tile_xT(xT):
    # [1024, 2048] -> [4 sblk, 128 part, 8 kc, 512]
    return np.ascontiguousarray(
        xT.reshape(KC, 128, 4, 512).transpose(2, 1, 0, 3))


def _tile_vT(vT):
    # [1024, 2048] -> [16 sc, 128 part, 8 kc, 128]
    return np.ascontiguousarray(
        vT.reshape(KC, 128, MT, 128).transpose(2, 1, 0, 3))


def _tile_w(w):
    # [1024, 256] -> [128, 8, 256]
    return np.ascontiguousarray(w.reshape(KC, 128, 256).transpose(1, 0, 2))


def _tile_mask(maskT_bf16):
    # [2048, 2048] -> [128, 16 m, 2048]
    return np.ascontiguousarray(
        maskT_bf16.reshape(MT, 128, S).transpose(1, 0, 2))


def kernel(q, k, v, mask, Wq, Wk, Wv, Wo, bo):
    global _BUILT
    if _BUILT is None:
        _BUILT = build_bass()
    nc = _BUILT

    BF = ml_dtypes.bfloat16
    q = np.asarray(q, np.float32)
    k = np.asarray(k, np.float32)
    v = np.asarray(v, np.float32)
    Wq = np.asarray(Wq, np.float32)
    Wk = np.asarray(Wk, np.float32)
    Wv = np.asarray(Wv, np.float32)
    Wo = np.asarray(Wo, np.float32)
    bo = np.asarray(bo, np.float32)
    mask = np.asarray(mask)

    cosT, sinT = _cos_sin_tables()
    perm = _rope_perm_cols()
    qTb = [_tile_xT(q[b].T.astype(BF)) for b in range(2)]
    kTb = [_tile_xT(k[b].T.astype(BF)) for b in range(2)]
    vTb = [_tile_vT(v[b].T.astype(BF)) for b in range(2)]
    maskTb = [_tile_mask(mask[b, 0].T.astype(BF)) for b in range(2)]

    in_maps = []
    for c in range(N_CORES):
        b = c // 4
        head_base = (c % 4) * 4
        cols = slice(head_base * 64, head_base * 64 + 256)
        in_maps.append({
            "qT": qTb[b], "kT": kTb[b], "vT": vTb[b],
            "wq": _tile_w(Wq[:, cols][:, perm].astype(BF)),
            "wk": _tile_w(Wk[:, cols][:, perm].astype(BF)),
            "wv": _tile_w(Wv[:, cols].astype(BF)),
            "wo": np.ascontiguousarray(
                Wo[cols, :].reshape(4, 64, DIM).transpose(1, 0, 2).astype(BF)),
            "cosT": cosT.astype(BF), "sinT": sinT.astype(BF),
            "maskT": maskTb[b],
        })

    kernel._last_in_maps = in_maps
    res = run_bass_kernel_spmd(nc, in_maps, core_ids=list(range(N_CORES)))
    out = np.zeros((2, S, DIM), np.float32)
    for c in range(N_CORES):
        out[c // 4] += res.results[c]["out_part"].astype(np.float32)
    out += bo[None, None, :]
    return out
